# revision 21
# baseline (speedup 1.0000x reference)
"""GNN message-passing (edge-conv style with segment-max aggregation) on 8 Trainium2 cores.

Sharding: edges are partitioned by destination-node range (core c owns nodes
[c*6250, (c+1)*6250)), so aggregation is core-local and no collective is needed.
Within a core, each node's incident edges are laid out rank-major ("sorted-ELL"):
nodes are ordered by descending in-degree (permutation pi); rank-row k holds the
k-th edge of every node with degree > k, so every rank-row is a dense prefix and
the per-node segment max becomes a sequence of dense elementwise-max updates.

Rank-rows are processed in a custom order (widest first, then all narrow rows,
then the remaining wide rows in descending width) so the per-row compute chains
of narrow rows hide under the long gathers of wide rows and the kernel ends on
a wide row with minimal drain. The rows are concatenated into one flat slot
stream gathered in 4096-index chunks (dma_gather of x[src] pair-rows, int16
indices via the src>>1 pair trick, parity-select on chip with a 0-stride
broadcast mask) -> PE transpose to feature-major -> h = W1a'@x_i + W1b@x_j
(+b1) -> LeakyReLU -> msg = W2@h -> elementwise max into the accumulator A.
Column ranges finalize (tanh(A+b2) + output DMA) as soon as the last row
touching them completes, overlapping the gather stream. Host re-permutes
columns and applies the empty-segment fill.
"""

import numpy as np

import concourse.bacc as bacc
import concourse.bass as bass
import concourse.mybir as mybir
import concourse.tile as tile
from concourse.bass_utils import run_bass_kernel_spmd
from concourse.masks import make_identity

N_NODES = 50000
N_EDGES = 800000
D = 64
NC = 8
NPC = N_NODES // NC          # 6250 nodes per core
XIW = ((NPC + 127) // 128) * 128   # 6272 padded node columns
P = 128
LEAKY = 0.01
GMAX = 4096  # max indices per dma_gather (HW-validated limit)
NARROW = 1024  # rows narrower than this are processed early (see module doc)

_CACHE = {}


def _roundup(a, m):
    return (a + m - 1) // m * m


def _row_order(W_list):
    """Process widest row first, then narrow rows, then wide rows descending."""
    order = sorted(range(len(W_list)), key=lambda k: (-W_list[k], k))
    wide = [k for k in order if W_list[k] >= NARROW]
    narrow = [k for k in order if W_list[k] < NARROW]
    if not wide:
        return order
    return [wide[0]] + narrow + wide[1:]


def _build_program(W_seq, tot_slots):
    """Build the (uniform across cores) Bass program; W_seq is the rank-row
    width sequence in processing order."""
    nc = bacc.Bacc("TRN2", target_bir_lowering=False, debug=False, num_devices=NC)
    dt = mybir.dt
    x2 = nc.dram_tensor("x2", [N_NODES // 2, 2 * D], dt.float32, kind="ExternalInput")
    idx = nc.dram_tensor("idx", [32, tot_slots // 16], dt.int16, kind="ExternalInput")
    par = nc.dram_tensor("par", [P, tot_slots // P], dt.uint8, kind="ExternalInput")
    WaT = nc.dram_tensor("WaT", [D + 1, D], dt.float32, kind="ExternalInput")
    WbT = nc.dram_tensor("WbT", [D, D], dt.float32, kind="ExternalInput")
    W2T = nc.dram_tensor("W2T", [D, D], dt.float32, kind="ExternalInput")
    b2c = nc.dram_tensor("b2c", [D, 1], dt.float32, kind="ExternalInput")
    onesd = nc.dram_tensor("onesd", [1, XIW], dt.float32, kind="ExternalInput")
    out = nc.dram_tensor("out", [D, XIW], dt.float32, kind="ExternalOutput")

    offs = np.concatenate([[0], np.cumsum(W_seq)]).astype(np.int64)
    S = int(offs[-1])            # main-stream slots (128-aligned)
    K = len(W_seq)

    with tile.TileContext(nc) as tc:
        with (
            tc.tile_pool(name="const", bufs=1) as cpool,
            tc.tile_pool(name="gath", bufs=3) as gpool,
            tc.tile_pool(name="work", bufs=3) as wpool,
            tc.tile_pool(name="vap", bufs=1) as vpool,
            tc.tile_pool(name="psum", bufs=2, space="PSUM") as ppool,
        ):
            # idx: DMA 16 rows, replicate to 128 partitions by doubling.
            # Split into two column pieces so early gathers start sooner.
            idx_sb = cpool.tile([P, tot_slots // 16], dt.int16, tag="idx")
            cut = (XIW + GMAX) // 16
            for c0, c1 in ((0, cut), (cut, tot_slots // 16)):
                nc.sync.dma_start(out=idx_sb[0:32, c0:c1], in_=idx[:, c0:c1])
                for g in (32, 64):
                    nc.vector.tensor_copy(
                        out=idx_sb[g : 2 * g, c0:c1], in_=idx_sb[0:g, c0:c1]
                    )

            ident = cpool.tile([P, P], dt.float32, tag="ident")
            make_identity(nc, ident[:])
            A = cpool.tile([D, XIW], dt.float32, tag="A")
            xiT = cpool.tile([D + 1, XIW], dt.float32r, tag="xiT")
            ones_f = cpool.tile([1, XIW], dt.float32, tag="ones")
            nc.sync.dma_start(out=ones_f[:], in_=onesd[:, :])
            nc.vector.tensor_copy(out=xiT[D : D + 1, :], in_=ones_f[:])

            def gather_chunk(goff, qw, pool, tag, mcap):
                """Gather slots [goff, goff+qw) (absolute, 128-aligned)."""
                m = _roundup(qw, P) // P
                g = pool.tile([P, mcap * 2 * D], dt.float32, tag="g" + tag)
                g3 = g[:].rearrange("p (m d) -> p m d", m=mcap)
                for q0 in range(0, qw, GMAX):
                    w = min(GMAX, qw - q0)
                    nc.gpsimd.dma_gather(
                        out_ap=g3[:, q0 // P : (q0 + w) // P, :],
                        in_ap=x2[:, :],
                        idxs_ap=idx_sb[:, (goff + q0) // 16 : (goff + q0 + w) // 16],
                        num_idxs=w,
                        num_idxs_reg=w,
                        elem_size=2 * D,
                        single_packet=False,
                    )
                pm_t = pool.tile([P, mcap], dt.uint8, tag="parm" + tag)
                nc.sync.dma_start(
                    out=pm_t[:, :m], in_=par[:, goff // P : goff // P + m]
                )
                return g3, pm_t

            def select_chunk(g3, pm_t, cs, mw):
                """edge-major x_j for blocks [cs, cs+mw) of this gather tile."""
                xsel = wpool.tile([P, 4 * D], dt.float32, tag="xsel")
                xsel3 = xsel[:].rearrange("p (m d) -> p m d", m=4)[:, :mw, :]
                nc.scalar.copy(out=xsel3, in_=g3[:, cs : cs + mw, 0:D])
                mp = pm_t[:, cs : cs + mw]
                mask_ap = bass.AP(mp.tensor, mp.offset, list(mp.ap) + [[0, D]])
                nc.vector.copy_predicated(
                    out=xsel3,
                    mask=mask_ap,
                    data=g3[:, cs : cs + mw, D : 2 * D],
                )
                return xsel3

            def transpose_chunks(xsel3, mw):
                pT = ppool.tile([D, 4 * P], dt.float32, tag="pT")
                for t in range(mw):
                    nc.tensor.transpose(
                        out=pT[:, t * P : (t + 1) * P],
                        in_=xsel3[:, t, :],
                        identity=ident[:],
                    )
                return pT


            # ---- x_i phase: gather x[pi[j]] feature-major into xiT rows
            # 0..63, using the same gather ring as the main stream
            for xc0 in range(0, XIW, GMAX):
                xqw = min(GMAX, XIW - xc0)
                gx3, pmx = gather_chunk(xc0, xqw, gpool, "", 32)
                for sub in range(0, xqw, 512):
                    w = min(512, xqw - sub)
                    mw = w // P
                    xv = select_chunk(gx3, pmx, sub // P, mw)
                    pT = transpose_chunks(xv, mw)
                    nc.scalar.copy(
                        out=xiT[0:D, xc0 + sub : xc0 + sub + w], in_=pT[:, :w]
                    )

            # weights (needed first by main-chunk compute)
            wa_f = cpool.tile([D + 1, D], dt.float32, tag="waf")
            nc.sync.dma_start(out=wa_f[:], in_=WaT[:, :])
            wa_sb = cpool.tile([D + 1, D], dt.float32r, tag="wa")
            nc.vector.tensor_copy(out=wa_sb[:], in_=wa_f[:])
            wb_sb = cpool.tile([D, D], dt.float32, tag="wb")
            nc.sync.dma_start(out=wb_sb[:], in_=WbT[:, :])
            w2_sb = cpool.tile([D, D], dt.float32, tag="w2")
            nc.sync.dma_start(out=w2_sb[:], in_=W2T[:, :])
            b2_sb = cpool.tile([D, 1], dt.float32, tag="b2")
            nc.sync.dma_start(out=b2_sb[:], in_=b2c[:, :])

            # VA[:, c] = W1a' @ x_i[c] + b1 -- row-invariant, computed once
            va = vpool.tile([D, XIW], dt.float16, tag="va")
            for v0 in range(0, XIW, 512):
                vw = min(512, XIW - v0)
                pv = ppool.tile([D, 512], dt.float32, tag="ph")
                nc.tensor.matmul(
                    out=pv[:, :vw], lhsT=wa_sb[:], rhs=xiT[:, v0 : v0 + vw],
                    start=True, stop=True,
                )
                nc.scalar.copy(out=va[:, v0 : v0 + vw], in_=pv[:, :vw])


            # ---- main stream: compute sub-chunks per gather chunk, clipped at
            # rank-row boundaries; fin ranges emitted when their last row ends.
            pieces_by_chunk = [[] for _ in range((S + GMAX - 1) // GMAX)]
            for i in range(K):
                s = int(offs[i])
                row_end = int(offs[i + 1])
                while s < row_end:
                    s1 = min(row_end, (s // GMAX + 1) * GMAX, s + 512)
                    pieces_by_chunk[s // GMAX].append((i, s, s1))
                    s = s1

            # columns [suffmax(i+1), W_seq[i]) finalize after row position i
            suffmax = [0] * (K + 1)
            for i in range(K - 1, -1, -1):
                suffmax[i] = max(suffmax[i + 1], int(W_seq[i]))
            fin_after = {}
            for i in range(K):
                r0, r1 = suffmax[i + 1], int(W_seq[i])
                if r1 > r0:
                    fin_after[i] = (r0, r1)

            def emit_fin(r0, r1):
                for f0 in range(r0, r1, 512):
                    fw = min(512, r1 - f0)
                    fin = wpool.tile([D, 512], dt.float32, tag="fin")
                    nc.scalar.activation(
                        out=fin[:, :fw],
                        in_=A[:, f0 : f0 + fw],
                        func=mybir.ActivationFunctionType.Tanh,
                        bias=b2_sb[:, 0:1],
                    )
                    nc.sync.dma_start(out=out[:, f0 : f0 + fw], in_=fin[:, :fw])

            if XIW > suffmax[0]:
                emit_fin(suffmax[0], XIW)

            last_piece_of_row = {}
            for ci, pieces in enumerate(pieces_by_chunk):
                for pj, (i, s0, s1) in enumerate(pieces):
                    last_piece_of_row[i] = (ci, pj)

            for ci, pieces in enumerate(pieces_by_chunk):
                c0 = ci * GMAX
                qw = min(GMAX, S - c0)
                g3, pm_t = gather_chunk(XIW + c0, qw, gpool, "", 32)
                views = []
                for pj, (i, s0, s1) in enumerate(pieces):
                    w = s1 - s0
                    mw = w // P
                    cs = (s0 - c0) // P
                    views.append(select_chunk(g3, pm_t, cs, mw))
                for pj, (i, s0, s1) in enumerate(pieces):
                    w = s1 - s0
                    mw = w // P
                    colr = s0 - int(offs[i])   # column offset within row i
                    xv = views[pj]
                    pT = transpose_chunks(xv, mw)
                    xjT = wpool.tile([D, 512], dt.float32, tag="xjT")
                    nc.scalar.copy(out=xjT[:, :w], in_=pT[:, :w])
                    ph = ppool.tile([D, 512], dt.float32, tag="ph")
                    nc.tensor.matmul(
                        out=ph[:, :w], lhsT=wb_sb[:], rhs=xjT[:, :w],
                        start=True, stop=True,
                    )
                    nc.vector.tensor_tensor(
                        out=ph[:, :w],
                        in0=ph[:, :w],
                        in1=va[:, colr : colr + w],
                        op=mybir.AluOpType.add,
                    )
                    h = xjT[:]
                    nc.scalar.activation(
                        out=h[:, :w],
                        in_=ph[:, :w],
                        func=mybir.ActivationFunctionType.Lrelu,
                        alpha=LEAKY,
                    )
                    pm = ppool.tile([D, 512], dt.float32, tag="pm")
                    nc.tensor.matmul(
                        out=pm[:, :w], lhsT=w2_sb[:], rhs=h[:, :w],
                        start=True, stop=True,
                    )
                    if i == 0:
                        # first (widest) row initializes A; no memset needed
                        nc.vector.tensor_copy(
                            out=A[:, colr : colr + w], in_=pm[:, :w]
                        )
                    else:
                        nc.vector.tensor_tensor(
                            out=A[:, colr : colr + w],
                            in0=A[:, colr : colr + w],
                            in1=pm[:, :w],
                            op=mybir.AluOpType.max,
                        )
                    if last_piece_of_row.get(i) == (ci, pj) and i in fin_after:
                        emit_fin(*fin_after[i])
    nc.compile()
    return nc


def _host_prep(x, edge_index, W1, b1, W2, b2):
    src = np.asarray(edge_index[0], dtype=np.int64)
    dst = np.asarray(edge_index[1], dtype=np.int64)
    x = np.ascontiguousarray(np.asarray(x, dtype=np.float32))

    per_core = []
    for c in range(NC):
        sel = (dst // NPC) == c
        s_c = src[sel]
        d_c = dst[sel] - c * NPC
        deg = np.bincount(d_c, minlength=NPC)
        pi = np.argsort(-deg, kind="stable")
        colpos = np.empty(NPC, np.int64)
        colpos[pi] = np.arange(NPC)
        order = np.argsort(d_c, kind="stable")
        ds = d_c[order]
        ss = s_c[order]
        starts = np.zeros(NPC + 1, np.int64)
        starts[1:] = np.cumsum(deg)
        rank = np.arange(len(ds), dtype=np.int64) - starts[ds]
        per_core.append(dict(deg=deg, pi=pi, colpos=colpos, ds=ds, ss=ss,
                             starts=starts, rank=rank))

    K = int(max(pc["deg"].max() for pc in per_core))
    # uniform rank-row widths across cores
    W_list = []
    for k in range(K):
        n_k = max(int((pc["deg"] > k).sum()) for pc in per_core)
        W_list.append(max(P, _roundup(n_k, P)))
    row_order = _row_order(W_list)           # processing position -> rank
    pos_of_rank = np.empty(K, np.int64)
    for i, k in enumerate(row_order):
        pos_of_rank[k] = i
    W_seq = [W_list[k] for k in row_order]
    offs = np.concatenate([[0], np.cumsum(W_seq)]).astype(np.int64)
    slots = int(offs[-1])
    tot = XIW + slots
    tot = _roundup(tot, 128 * 16)  # keep /16 and /128 layouts aligned
    pad_tail = tot - (XIW + slots)

    in_maps = []
    x2 = x.reshape(N_NODES // 2, 2 * D)
    W1 = np.asarray(W1, dtype=np.float32)
    b1 = np.asarray(b1, dtype=np.float32)
    W2 = np.asarray(W2, dtype=np.float32)
    b2 = np.asarray(b2, dtype=np.float32)
    W1a, W1b = W1[:, :D], W1[:, D:]
    WaT = np.ascontiguousarray(
        np.concatenate([(W1a - W1b).T, b1[None, :]], axis=0), dtype=np.float32
    )
    WbT = np.ascontiguousarray(W1b.T, dtype=np.float32)
    W2T = np.ascontiguousarray(W2.T, dtype=np.float32)
    b2c = np.ascontiguousarray(b2[:, None], dtype=np.float32)

    for c in range(NC):
        pc = per_core[c]
        deg, pi, colpos = pc["deg"], pc["pi"], pc["colpos"]
        ds, ss, starts, rank = pc["ds"], pc["ss"], pc["starts"], pc["rank"]

        first_src = np.zeros(NPC, np.int64)
        nz = deg > 0
        first_src[nz] = ss[starts[:-1][nz]]
        dup_by_col = np.zeros(XIW, np.int64)
        dup_by_col[colpos] = first_src

        src_slot = np.empty(slots, np.int64)
        for i in range(K):
            src_slot[offs[i] : offs[i + 1]] = dup_by_col[: W_seq[i]]
        src_slot[offs[pos_of_rank[rank]] + colpos[ds]] = ss

        xi_global = np.full(XIW, c * NPC, np.int64)
        xi_global[:NPC] = c * NPC + pi

        full_slots = np.concatenate(
            [xi_global, src_slot, np.zeros(pad_tail, np.int64)]
        )
        idx16 = np.ascontiguousarray(
            np.tile((full_slots >> 1).astype(np.int16).reshape(-1, 16).T, (2, 1))
        )
        parity = np.ascontiguousarray(
            (full_slots & 1).astype(np.uint8).reshape(-1, P).T
        )
        in_maps.append({
            "x2": x2, "idx": idx16, "par": parity,
            "WaT": WaT, "WbT": WbT, "W2T": W2T, "b2c": b2c,
            "onesd": np.ones((1, XIW), np.float32),
        })

    meta = dict(W_seq=tuple(W_seq), tot=tot, per_core=per_core)
    return in_maps, meta


def _run(inputs, trace=False):
    in_maps, meta = _host_prep(
        inputs["x"], inputs["edge_index"], inputs["W1"], inputs["b1"],
        inputs["W2"], inputs["b2"],
    )
    key = (meta["W_seq"], meta["tot"])
    if key not in _CACHE:
        _CACHE[key] = _build_program(list(meta["W_seq"]), meta["tot"])
    nc = _CACHE[key]
    res = run_bass_kernel_spmd(nc, in_maps, core_ids=list(range(NC)), trace=trace)

    out = np.zeros((N_NODES, D), np.float32)
    for c in range(NC):
        pc = meta["per_core"][c]
        r = res.results[c]["out"]  # [64, XIW]
        out[c * NPC + pc["pi"]] = r[:, :NPC].T
        empty = np.nonzero(pc["deg"] == 0)[0]
        out[c * NPC + empty] = 0.0
    return out, res


def kernel(**inputs) -> np.ndarray:
    out, _ = _run(inputs, trace=False)
    return out
